# revision 3
# baseline (speedup 1.0000x reference)
"""Varlen causal GQA attention on 8 TRN2 NeuronCores.

Problem: 32 q heads, 8 kv heads, head_dim 128, ragged batch (cu_seqlens),
f32. Sharded by KV-head group: core c owns kv head c and q heads
4c..4c+3 — fully data-independent across cores, no collectives.

Per core the kernel computes, for each of its 4 q heads, blockwise
causal attention over each sequence:
    S^T[k, q] = (K_j)^T.T @ Q^T        (float32r matmuls, d contracted)
    P^T = exp(S^T * scale)             (ScalarE, causal mask added on DVE)
    O^T[d, q] += V_j.T? no: lhsT=V_j (natural [k, d]) rhs=P^T  (PSUM accum)
    sums[1, q] += ones.T @ P^T         (PSUM accum)
All transposes (Q^T, K^T on the way in; O^T -> O and the softmax
division on the way out) happen host-side in numpy, so the device
executes only matmuls, one exp pass, and mask adds.
"""

import math
import os
import sys

sys.path.insert(0, "/opt/trn_rl_repo")

import numpy as np

NUM_HEADS = 32
NUM_KV_HEADS = 8
HEAD_DIM = 128
HEADS_PER_CORE = NUM_HEADS // NUM_KV_HEADS  # 4
N_CORES = 8
BLK = 128
GROUP = 512
SCALE = 1.0 / math.sqrt(HEAD_DIM)

_GRAPH_CACHE = {}


def _build_graph(seq_blocks):
    """Build the SPMD Bacc graph for padded per-seq block counts."""
    from concourse import bacc
    import concourse.mybir as mybir
    from concourse.tile import TileContext

    f32 = mybir.dt.float32
    f32r = mybir.dt.float32r
    T = sum(seq_blocks) * BLK
    n_blocks_total = T // BLK

    nc = bacc.Bacc("TRN2", target_bir_lowering=False, debug=False,
                   num_devices=N_CORES)

    qT_ext = [
        nc.declare_dram_parameter(f"qT{h}", [BLK, T], f32, isOutput=False)
        for h in range(HEADS_PER_CORE)
    ]
    kT_ext = nc.declare_dram_parameter("kT", [BLK, T], f32, isOutput=False)
    v_ext = nc.declare_dram_parameter("v", [T, HEAD_DIM], f32, isOutput=False)
    mask_ext = nc.declare_dram_parameter("mask", [BLK, BLK], f32, isOutput=False)
    oT_ext = [
        nc.declare_dram_parameter(f"oT{h}", [BLK, T], f32, isOutput=True)
        for h in range(HEADS_PER_CORE)
    ]
    sums_ext = [
        nc.declare_dram_parameter(f"sums{h}", [1, T], f32, isOutput=True)
        for h in range(HEADS_PER_CORE)
    ]

    with TileContext(nc) as tc:
        with (
            tc.tile_pool(name="persist", bufs=1) as persist,
            tc.tile_pool(name="qstage", bufs=2) as qstage,
            tc.tile_pool(name="qr", bufs=2) as qr_pool,
            tc.tile_pool(name="ot", bufs=2) as ot_pool,
            tc.tile_pool(name="p", bufs=3) as p_pool,
            tc.tile_pool(name="ps_s", bufs=3, space="PSUM") as ps_s,
            tc.tile_pool(name="ps_o", bufs=2, space="PSUM") as ps_o,
            tc.tile_pool(name="ps_sum", bufs=2, space="PSUM") as ps_sum,
        ):
            # ---- load + round shared tensors (k^T, v, mask, ones)
            kT_sb = persist.tile([BLK, T], f32)
            v_sb = persist.tile([BLK, n_blocks_total, HEAD_DIM], f32)
            mask_sb = persist.tile([BLK, BLK], f32)
            nc.sync.dma_start(kT_sb[:], kT_ext[:])
            nc.sync.dma_start(
                v_sb[:], v_ext[:].rearrange("(j p) d -> p j d", p=BLK)
            )
            nc.sync.dma_start(mask_sb[:], mask_ext[:])

            ones_f = persist.tile([BLK, 1], f32)
            nc.vector.memset(ones_f[:], 1.0)
            ones_r = persist.tile([BLK, 1], f32r)
            nc.gpsimd.tensor_copy(ones_r[:], ones_f[:])

            kT_r = persist.tile([BLK, T], f32r)
            v_r = persist.tile([BLK, n_blocks_total, HEAD_DIM], f32r)
            # chunked rounding copies so compute can start early
            CH = 1024
            for c0 in range(0, T, CH):
                c1 = min(T, c0 + CH)
                nc.gpsimd.tensor_copy(kT_r[:, c0:c1], kT_sb[:, c0:c1])
            for b0 in range(0, n_blocks_total, 8):
                b1 = min(n_blocks_total, b0 + 8)
                nc.gpsimd.tensor_copy(v_r[:, b0:b1, :], v_sb[:, b0:b1, :])

            for h in range(HEADS_PER_CORE):
                q_sb = qstage.tile([BLK, T], f32, tag="qstage")
                nc.sync.dma_start(q_sb[:], qT_ext[h][:])
                qT_r = qr_pool.tile([BLK, T], f32r, tag="qr")
                for c0 in range(0, T, CH):
                    c1 = min(T, c0 + CH)
                    nc.gpsimd.tensor_copy(qT_r[:, c0:c1], q_sb[:, c0:c1])

                ot_stage = ot_pool.tile([BLK, T], f32, tag="ot")
                sums_stage = ot_pool.tile([1, T], f32, tag="sums")

                seq_off = 0
                for nblk in seq_blocks:
                    Ls = nblk * BLK
                    for g in range((Ls + GROUP - 1) // GROUP):
                        Q0 = g * GROUP
                        W = min(GROUP, Ls - Q0)
                        jmax = (Q0 + W) // BLK - 1
                        oT_ps = ps_o.tile([BLK, GROUP], f32, tag="ot_ps")
                        sums_ps = ps_sum.tile([1, GROUP], f32, tag="sums_ps")
                        for j in range(jmax + 1):
                            cs = max(0, BLK * j - Q0)
                            N = W - cs
                            s_ps = ps_s.tile([BLK, GROUP], f32, tag="s_ps")
                            nc.tensor.matmul(
                                s_ps[:, :N],
                                kT_r[:, seq_off + j * BLK : seq_off + (j + 1) * BLK],
                                qT_r[:, seq_off + Q0 + cs : seq_off + Q0 + cs + N],
                                start=True,
                                stop=True,
                            )
                            if BLK * j >= Q0:  # diagonal block: causal mask
                                nc.vector.tensor_add(
                                    s_ps[:, :BLK], s_ps[:, :BLK], mask_sb[:]
                                )
                            p_r = p_pool.tile([BLK, GROUP], f32r, tag="p")
                            nc.scalar.activation(
                                p_r[:, :N],
                                s_ps[:, :N],
                                mybir.ActivationFunctionType.Exp,
                                scale=SCALE,
                            )
                            nc.tensor.matmul(
                                oT_ps[:, cs : cs + N],
                                v_r[:, seq_off // BLK + j, :],
                                p_r[:, :N],
                                start=(j == 0),
                                stop=(j == jmax),
                            )
                            nc.tensor.matmul(
                                sums_ps[:, cs : cs + N],
                                ones_r[:],
                                p_r[:, :N],
                                start=(j == 0),
                                stop=(j == jmax),
                            )
                        nc.vector.tensor_copy(
                            ot_stage[:, seq_off + Q0 : seq_off + Q0 + W],
                            oT_ps[:, :W],
                        )
                        nc.vector.tensor_copy(
                            sums_stage[:, seq_off + Q0 : seq_off + Q0 + W],
                            sums_ps[:, :W],
                        )
                    seq_off += Ls

                nc.sync.dma_start(oT_ext[h][:], ot_stage[:])
                nc.sync.dma_start(sums_ext[h][:], sums_stage[:])

    nc.finalize()
    return nc


def _install_ntff_hook():
    """Shim antenv.axon_hooks (absent in this container) so trace=True can
    reach the terminal's NRT profiler via libaxon_pjrt.so ctypes."""
    import types

    if "antenv.axon_hooks" in sys.modules:
        return
    import antenv
    from concourse import bass_utils

    mod = types.ModuleType("antenv.axon_hooks")
    state = {"hook": None}
    mod.set_axon_ntff_profile_hook = lambda h: state.__setitem__("hook", h)
    mod.get_axon_ntff_profile_hook = lambda: state["hook"]
    sys.modules["antenv.axon_hooks"] = mod
    antenv.axon_hooks = mod
    bass_utils.upload_artifacts = lambda tmpdir: tmpdir  # zero-egress container
    try:
        if "/root/.axon_site" not in sys.path:
            sys.path.insert(0, "/root/.axon_site")
        from trn_agent_boot.trn_boot import _ntff_profile_via_ctypes

        mod.set_axon_ntff_profile_hook(
            _ntff_profile_via_ctypes("/opt/axon/libaxon_pjrt.so")
        )
    except Exception:
        pass


def kernel(q, k, v, cu_seqlens, max_seqlen):
    from concourse import bass_utils

    q = np.asarray(q, dtype=np.float32)
    k = np.asarray(k, dtype=np.float32)
    v = np.asarray(v, dtype=np.float32)
    cu = np.asarray(cu_seqlens, dtype=np.int64)
    T_host = q.shape[0]
    lengths = np.diff(cu).astype(np.int64)
    nblocks = [int((L + BLK - 1) // BLK) for L in lengths]
    T_pad = sum(nblocks) * BLK

    # host -> padded device token index map (valid tokens only)
    dev_idx = np.zeros(T_host, dtype=np.int64)
    pad_off = 0
    for s, L in enumerate(lengths):
        L = int(L)
        dev_idx[cu[s] : cu[s] + L] = pad_off + np.arange(L)
        pad_off += nblocks[s] * BLK

    qp = np.zeros((T_pad, NUM_HEADS * HEAD_DIM), np.float32)
    kp = np.zeros((T_pad, NUM_KV_HEADS * HEAD_DIM), np.float32)
    vp = np.zeros((T_pad, NUM_KV_HEADS * HEAD_DIM), np.float32)
    qp[dev_idx] = q
    kp[dev_idx] = k
    vp[dev_idx] = v

    mask = np.where(
        np.arange(BLK)[:, None] <= np.arange(BLK)[None, :], 0.0, -1e30
    ).astype(np.float32)

    key = tuple(nblocks)
    if key not in _GRAPH_CACHE:
        _GRAPH_CACHE[key] = _build_graph(key)
    nc = _GRAPH_CACHE[key]

    in_maps = []
    for c in range(N_CORES):
        m = {"mask": mask}
        m["kT"] = np.ascontiguousarray(kp[:, c * HEAD_DIM : (c + 1) * HEAD_DIM].T)
        m["v"] = np.ascontiguousarray(vp[:, c * HEAD_DIM : (c + 1) * HEAD_DIM])
        for h in range(HEADS_PER_CORE):
            gh = c * HEADS_PER_CORE + h
            m[f"qT{h}"] = np.ascontiguousarray(
                qp[:, gh * HEAD_DIM : (gh + 1) * HEAD_DIM].T
            )
        in_maps.append(m)

    trace = bool(os.environ.get("BASS_TRACE"))
    if trace:
        _install_ntff_hook()
    res = bass_utils.run_bass_kernel_spmd(
        nc, in_maps, core_ids=list(range(N_CORES)), trace=trace
    )
    if trace and res.exec_time_ns is not None:
        print(f"HW exec time: {res.exec_time_ns} ns")
        if res.instructions_and_trace is not None:
            print(f"trace: {res.instructions_and_trace[1]}")

    out = np.empty((T_host, NUM_HEADS * HEAD_DIM), np.float32)
    for c in range(N_CORES):
        r = res.results[c]
        for h in range(HEADS_PER_CORE):
            gh = c * HEADS_PER_CORE + h
            oT = r[f"oT{h}"]  # [128, T_pad] unnormalized
            sums = r[f"sums{h}"][0]  # [T_pad]
            o = (oT[:, dev_idx] / sums[dev_idx][None, :]).T  # [T_host, 128]
            out[:, gh * HEAD_DIM : (gh + 1) * HEAD_DIM] = o
    return out
